# revision 6
# baseline (speedup 1.0000x reference)
"""Multi-head attention (B=4, S=2048, D=1024, 16 heads x 64) on 8 TRN2 NeuronCores.

Sharding: core c -> (batch b = c//2, head-group g = c%2) -- data parallel on
batch, tensor parallel over heads (8 heads / 512 projection columns per core).
Each core computes its 8 heads' attention and a partial output projection
(contraction over its 512 O-columns); the host sums the two partials per batch
and adds the output bias.

On-device layouts (all transposed so no on-device transposes are needed):
  xt = X[b]^T               [1024(k), 2048(m)]   (host-transposed, bf16)
  Q^T, K^T                  [512(n), 2048(m)] as sbuf [128, 4nt, 2048]
                            (head h at nt=h//2, partition base (h%2)*64)
  V (augmented)             [2048(j), 8 heads x (64 cols | ones col)] -- the
                            ones column yields softmax row-sums in the U matmul
  S^T = K_h @ Q_h^T         psum [128(j), 1024(i-half)] per (head, jtile);
                            head pairs run as K=64 row-tiled matmuls
  E^T = exp(S^T/32)         bf16 sbuf (scale fused into ACT; no max-subtract:
                            |scores| <= ~3 for randn inputs)
  U^T_aug = V_aug^T @ E^T   psum [65, 1024] accumulated over 16 jtiles
  O^T = U^T * (1/rowsum)    bf16 [64, 8 heads, 2048]; rowsum broadcast via a
                            K=1 PE matmul from partition 64, DVE reciprocal
  Y = O @ Wo_rows           per-head K=64 matmuls, accumulated over 8 heads;
                            bo added on host together with the pairwise sum
"""

import numpy as np
import ml_dtypes

import concourse.bass as bass
from concourse import bacc
import concourse.tile as tile
import concourse.mybir as mybir
from concourse.bass_utils import run_bass_kernel_spmd

F32 = mybir.dt.float32
BF16 = mybir.dt.bfloat16
AF = mybir.ActivationFunctionType
BF = ml_dtypes.bfloat16

B, S, D = 4, 2048, 1024
HEADS, HD = 16, 64
G = 2                 # head groups (tensor-parallel factor)
HPC = HEADS // G      # 8 heads per core
NC = D // G           # 512 projection columns per core
NT = NC // 128        # 4 partition tiles of Q^T/K^T
KT = D // 128         # 8 contraction tiles
MT = S // 128         # 16 seq tiles
PAIRS = NT            # 4 head pairs (even head base 0, odd head base 64)
VA = HD + 1           # 65: augmented V width per head [V64|ones]
VW = HPC * VA         # 520
IH = 2                # i-halves
IHW = S // IH         # 1024
SC = 1.0 / 32.0       # reference divides scores by head_dim/2 == 32

_CACHED_NC = None


def _build():
    nc = bacc.Bacc("TRN2", target_bir_lowering=False, debug=False)
    xt_d = nc.dram_tensor("xt", [D, S], BF16, kind="ExternalInput").ap()
    wq_d = nc.dram_tensor("wq", [D, NC], BF16, kind="ExternalInput").ap()
    wk_d = nc.dram_tensor("wk", [D, NC], BF16, kind="ExternalInput").ap()
    wv_d = nc.dram_tensor("wv", [D, VW], BF16, kind="ExternalInput").ap()
    wo_d = nc.dram_tensor("wo", [NC, D], BF16, kind="ExternalInput").ap()
    bq_d = nc.dram_tensor("bq", [NT, 128], F32, kind="ExternalInput").ap()
    bk_d = nc.dram_tensor("bk", [NT, 128], F32, kind="ExternalInput").ap()
    bv_d = nc.dram_tensor("bv", [1, VW], BF16, kind="ExternalInput").ap()
    y_d = nc.dram_tensor("y", [S, D], F32, kind="ExternalOutput").ap()

    with tile.TileContext(nc) as tc:
        _body(nc, tc, xt_d, wq_d, wk_d, wv_d, wo_d, bq_d, bk_d, bv_d, y_d)
    nc.compile()
    return nc


def _body(nc, tc, xt_d, wq_d, wk_d, wv_d, wo_d, bq_d, bk_d, bv_d, y_d):
    from contextlib import ExitStack
    with ExitStack() as ctx:
        persist = ctx.enter_context(tc.tile_pool(name="persist", bufs=1))
        qt = persist.tile([128, NT, S], BF16)
        kt = persist.tile([128, NT, S], BF16)
        v_ = persist.tile([128, MT, VW], BF16)
        ot = persist.tile([64, HPC, S], BF16)
        ones = persist.tile([128, 128], BF16)
        bq_sb = persist.tile([128, NT], F32)
        bk_sb = persist.tile([128, NT], F32)
        bv_sb = persist.tile([1, VW], BF16)
        wo_sb = persist.tile([64, HPC, D], BF16)

        nc.vector.memset(ones[:], 1.0)

        for n in range(NT):
            nc.sync.dma_start(bq_sb[:, n : n + 1], bq_d[n, :].unsqueeze(-1))
            nc.sync.dma_start(bk_sb[:, n : n + 1], bk_d[n, :].unsqueeze(-1))
        nc.sync.dma_start(bv_sb[:], bv_d[:])
        for h in range(HPC):
            nc.sync.dma_start(wo_sb[:, h, :], wo_d[h * HD : (h + 1) * HD, :])

        xw = ctx.enter_context(tc.tile_pool(name="xw", bufs=1))
        xt_sb = xw.tile([128, KT, S], BF16)
        for k in range(KT):
            nc.sync.dma_start(xt_sb[:, k, :], xt_d[k * 128 : (k + 1) * 128, :])

        wpool = ctx.enter_context(tc.tile_pool(name="wpool", bufs=2))

        # ---------------- projections ----------------
        with tc.tile_pool(name="proj_ps", bufs=4, space="PSUM") as pps:
            # Q^T and K^T:  out[n,m] = sum_k W[k,n] * X^T[k,m]
            for w_d, b_sb, dst in ((wq_d, bq_sb, qt), (wk_d, bk_sb, kt)):
                w_sb = wpool.tile([128, KT, VW], BF16, tag="w")
                for k in range(KT):
                    nc.sync.dma_start(w_sb[:, k, :NC], w_d[k * 128 : (k + 1) * 128, :])
                for n in range(NT):
                    for mc in range(NT):
                        ps = pps.tile([128, 512], F32, tag="pp", bufs=4)
                        for k in range(KT):
                            nc.tensor.matmul(
                                ps[:],
                                w_sb[:, k, n * 128 : (n + 1) * 128],
                                xt_sb[:, k, mc * 512 : (mc + 1) * 512],
                                start=(k == 0),
                                stop=(k == KT - 1),
                            )
                        nc.vector.tensor_scalar_add(
                            dst[:, n, mc * 512 : (mc + 1) * 512], ps[:], b_sb[:, n : n + 1]
                        )

            # V: out[m, n'] = sum_k X^T[k,m] * Wv_aug[k,n'] + bv_aug (ones-row trick)
            wv_sb = wpool.tile([128, KT, VW], BF16, tag="w")
            for k in range(KT):
                nc.sync.dma_start(wv_sb[:, k, :], wv_d[k * 128 : (k + 1) * 128, :])
            for m in range(MT):
                ps = pps.tile([128, VW], F32, tag="pv", bufs=2)
                for lo, hi in ((0, 512), (512, VW)):
                    for k in range(KT):
                        nc.tensor.matmul(
                            ps[:, lo:hi],
                            xt_sb[:, k, m * 128 : (m + 1) * 128],
                            wv_sb[:, k, lo:hi],
                            start=(k == 0),
                            stop=False,
                        )
                    nc.tensor.matmul(
                        ps[:, lo:hi], ones[0:1, :128], bv_sb[0:1, lo:hi],
                        start=False, stop=True,
                    )
                nc.scalar.copy(v_[:, m, :], ps[:])

        # ---------------- attention + output projection ----------------
        epool = ctx.enter_context(tc.tile_pool(name="epool", bufs=3))
        rspool = ctx.enter_context(tc.tile_pool(name="rspool", bufs=2))
        bpool = ctx.enter_context(tc.tile_pool(name="bpool", bufs=2))
        ypool = ctx.enter_context(tc.tile_pool(name="ypool", bufs=3))
        spool = ctx.enter_context(tc.tile_pool(name="spool", bufs=2, space="PSUM"))
        upool = ctx.enter_context(tc.tile_pool(name="upool", bufs=2, space="PSUM"))

        NIC = IHW // 512

        for ih in range(IH):
            i0 = ih * IHW
            for t in range(PAIRS):
                hA, hB = 2 * t, 2 * t + 1
                uA = upool.tile([128, IHW], F32, tag="u")
                uB = upool.tile([128, IHW], F32, tag="u")
                for j in range(MT):
                    js = slice(j * 128, (j + 1) * 128)
                    sA = spool.tile([128, IHW], F32, tag="s")
                    sB = spool.tile([128, IHW], F32, tag="s")
                    for ic in range(NIC):
                        cs = slice(ic * 512, (ic + 1) * 512)
                        qs = slice(i0 + ic * 512, i0 + (ic + 1) * 512)
                        nc.tensor.matmul(sA[:, cs], kt[0:64, t, js], qt[0:64, t, qs],
                                         start=True, stop=True)
                        nc.tensor.matmul(sB[:, cs], kt[64:128, t, js], qt[64:128, t, qs],
                                         start=True, stop=True)
                    eA = epool.tile([128, IHW], BF16, tag="e")
                    nc.scalar.activation(eA[:], sA[:], AF.Exp, scale=SC)
                    eB = epool.tile([128, IHW], BF16, tag="e")
                    nc.scalar.activation(eB[:], sB[:], AF.Exp, scale=SC)
                    for ic in range(NIC):
                        cs = slice(ic * 512, (ic + 1) * 512)
                        nc.tensor.matmul(
                            uA[0:VA, cs], v_[:, j, hA * VA : (hA + 1) * VA], eA[:, cs],
                            start=(j == 0), stop=(j == MT - 1),
                        )
                        nc.tensor.matmul(
                            uB[0:VA, cs], v_[:, j, hB * VA : (hB + 1) * VA], eB[:, cs],
                            start=(j == 0), stop=(j == MT - 1),
                        )
                # epilogue: O_h^T = U^T[0:64] * bcast(1 / U^T[64])
                for u, h in ((uA, hA), (uB, hB)):
                    rs = rspool.tile([128, IHW], BF16, tag="rs")
                    nc.vector.tensor_copy(rs[64:65, :], u[64:65, :])
                    b_ps = spool.tile([128, IHW], F32, tag="s")
                    for ic in range(NIC):
                        cs = slice(ic * 512, (ic + 1) * 512)
                        nc.tensor.matmul(b_ps[0:64, cs], ones[64:65, 0:64],
                                         rs[64:65, cs], start=True, stop=True,
                                         tile_position=(64, 0))
                    bb = bpool.tile([64, IHW], F32, tag="b")
                    nc.vector.reciprocal_approx_fast(out=bb[:, :], in_=b_ps[0:64, :])
                    nc.vector.tensor_mul(ot[:, h, i0 : i0 + IHW], u[0:64, :], bb[:, :])

            # output projection rows for this i-half
            for it in range(ih * MT // 2, (ih + 1) * MT // 2):
                y_sb = ypool.tile([128, D], F32, tag="y")
                for cc in range(2):
                    y_ps = upool.tile([128, 512], F32, tag="u")
                    for h in range(HPC):
                        nc.tensor.matmul(
                            y_ps[:], ot[:, h, it * 128 : (it + 1) * 128],
                            wo_sb[:, h, cc * 512 : (cc + 1) * 512],
                            start=(h == 0), stop=(h == HPC - 1),
                        )
                    nc.scalar.copy(y_sb[:, cc * 512 : (cc + 1) * 512], y_ps[:])
                nc.sync.dma_start(y_d[it * 128 : (it + 1) * 128, :], y_sb[:])


def _prep_core_inputs(inputs, b, g):
    cols = slice(g * NC, (g + 1) * NC)
    x = np.asarray(inputs["inputs"], dtype=np.float32)
    wv = np.asarray(inputs["Wv"], dtype=np.float32)[:, cols]
    bv = np.asarray(inputs["bv"], dtype=np.float32)[cols]
    wv_aug = np.zeros((D, VW), dtype=np.float32)
    bv_aug = np.zeros((1, VW), dtype=np.float32)
    for h in range(HPC):
        wv_aug[:, h * VA : h * VA + HD] = wv[:, h * HD : (h + 1) * HD]
        bv_aug[0, h * VA : h * VA + HD] = bv[h * HD : (h + 1) * HD]
        bv_aug[0, h * VA + HD] = 1.0
    return {
        "xt": np.ascontiguousarray(x[b].T).astype(BF),
        "wq": np.asarray(inputs["Wq"], dtype=np.float32)[:, cols].astype(BF),
        "wk": np.asarray(inputs["Wk"], dtype=np.float32)[:, cols].astype(BF),
        "wv": wv_aug.astype(BF),
        "wo": np.asarray(inputs["Wo"], dtype=np.float32)[cols, :].astype(BF),
        "bq": np.asarray(inputs["bq"], dtype=np.float32)[cols].reshape(NT, 128).copy(),
        "bk": np.asarray(inputs["bk"], dtype=np.float32)[cols].reshape(NT, 128).copy(),
        "bv": bv_aug.astype(BF),
    }


def _get_nc():
    global _CACHED_NC
    if _CACHED_NC is None:
        _CACHED_NC = _build()
    return _CACHED_NC


def _run(inputs, trace=False, trace_kwargs=None):
    nc = _get_nc()
    in_maps = [_prep_core_inputs(inputs, c // G, c % G) for c in range(8)]
    res = run_bass_kernel_spmd(nc, in_maps, list(range(8)), trace=trace,
                               **(trace_kwargs or {}))
    bo = np.asarray(inputs["bo"], dtype=np.float32)
    out = np.empty((B, S, D), dtype=np.float32)
    for b in range(B):
        out[b] = res.results[2 * b]["y"] + res.results[2 * b + 1]["y"] + bo
    return out, res


def kernel(**inputs) -> np.ndarray:
    out, _ = _run(inputs)
    return out
